# revision 32
# baseline (speedup 1.0000x reference)
"""BRGCN (2-layer relational GCN with bilinear attention) on 8 TRN2 NeuronCores.

Sharding: edges are sharded by dst node (6250 nodes/core) so the
(relation, dst) segment softmax and the z scatter are fully core-local; node
features are replicated. Per core the edges are sorted by t = dst_local*8+rel,
binned into 391 blocks of 128 t-rows and padded into 128-edge K-tiles (tile
counts uniform across cores -> one SPMD program; the kernel is compiled per
call so all binning is static). The z scatter is a one-hot matmul with
Sfac[e,t] = (iota[t]==t_e)*F[t], F[t]=exp(si[t]/2); the fused accum_out of that
op delivers the per-edge dst attention term si, and F cancels in the z/d
normalization. Per-edge src features come from one dma_gather of bf16 rows
[1 | h | sj | pad]. The tail (per-relation q/k/v, psi, channel softmax) is
node-parallel; one AllGather bridges layer1 -> layer2. All matmul/DVE
traffic is bf16 (rel err ~4e-3, within the 2e-2 gate); accumulation stays
fp32 in PSUM.
"""
import numpy as np

R = 8
N = 50000
E = 800000
NCORES = 8
Nc = N // NCORES            # 6250
TB = 128
RNc = R * Nc                # 50000
RNc_pad = ((RNc + TB - 1) // TB) * TB   # 50048
NBLK = RNc_pad // TB        # 391
SPLITS = [0, 16672, 33344, N]
NG = 3
C1, C2 = 128, 64
ROW1, ROW2 = 256, 128       # gathered row length in bf16 elems (512B / 256B)
EPS = 1e-30
WBLK = 8                    # blocks per gather window
NW = (NBLK + WBLK - 1) // WBLK


def _host_prep(edge_index, edge_type):
    src = edge_index[0].astype(np.int64)
    dst = edge_index[1].astype(np.int64)
    et = edge_type.astype(np.int64)
    percore = []
    counts = np.zeros((NCORES, NBLK, NG), np.int64)
    for m in range(NCORES):
        sel = (dst // Nc) == m
        s, d, r = src[sel], dst[sel], et[sel]
        t = (d - m * Nc) * 8 + r
        blk = t // TB
        grp = np.digitize(s, SPLITS[1:NG])
        order = np.lexsort((t, grp, blk))
        s, t, r, blk, grp = s[order], t[order], r[order], blk[order], grp[order]
        percore.append((s, t, r, blk, grp))
        for g in range(NG):
            np.add.at(counts[m, :, g], blk[grp == g], 1)
    tiles = np.maximum(-(-counts.max(0) // 128), 1)   # [NBLK, NG]

    # column maps, shared across cores
    colmap = {}
    call_cols = np.zeros((NW, NG), np.int64)
    for w in range(NW):
        for g in range(NG):
            c = 0
            for b in range(w * WBLK, min((w + 1) * WBLK, NBLK)):
                for k in range(int(tiles[b, g])):
                    colmap[(b, g, k)] = c
                    c += 1
            call_cols[w, g] = c
    call_off = np.zeros((NW, NG), np.int64)
    acc = 0
    for w in range(NW):
        for g in range(NG):
            call_off[w, g] = acc
            acc += call_cols[w, g]
    ncols = int(acc)
    gcols = [int(call_cols[:, g].sum()) for g in range(NG)]
    gcol_off = np.zeros((NW, NG), np.int64)
    for g in range(NG):
        a = 0
        for w in range(NW):
            gcol_off[w, g] = a
            a += call_cols[w, g]

    tabs = []
    for m in range(NCORES):
        s, t, r, blk, grp = percore[m]
        tcol = np.zeros(ncols * 128, np.float32)
        mask = np.full(ncols * 128, -1e30, np.float32)
        idxs = [np.zeros(gcols[g] * 128, np.int64) for g in range(NG)]
        m8s = [np.zeros((gcols[g] * 128, 8), np.float32) for g in range(NG)]
        for b in range(NBLK):
            w = b // WBLK
            for g in range(NG):
                bm = (blk == b) & (grp == g)
                cnt = int(bm.sum())
                c0 = colmap[(b, g, 0)]
                o = (int(call_off[w, g]) + c0) * 128
                tcol[o:o + cnt] = (t[bm] - b * TB).astype(np.float32)
                mask[o:o + cnt] = 0.0
                io = (int(gcol_off[w, g]) + c0) * 128
                idxs[g][io:io + cnt] = s[bm] - SPLITS[g]
                m8s[g][np.arange(io, io + cnt), r[bm]] = 1.0

        def wrap16(a):
            ar = a.reshape(-1, 16).T
            return np.ascontiguousarray(np.tile(ar, (8, 1))).astype(np.int16)

        tabs.append(dict(
            tcol=np.ascontiguousarray(tcol.reshape(-1, 128).T),
            mask=np.ascontiguousarray(mask.reshape(-1, 128).T),
            idx=[wrap16(idxs[g]) for g in range(NG)],
            msk8=[np.ascontiguousarray(
                m8s[g].reshape(-1, 128, 8).transpose(1, 0, 2).reshape(128, -1))
                for g in range(NG)],
        ))
    meta = dict(tiles=tiles, colmap=colmap, call_cols=call_cols,
                call_off=call_off, gcol_off=gcol_off, ncols=ncols, gcols=gcols)
    return tabs, meta


def _emit(nc, tc, bass, mybir, meta, H):
    import os
    _stop = os.environ.get("BRGCN_STOP", "")
    dt = mybir.dt
    f32 = dt.float32
    bf = dt.bfloat16
    A = mybir.AluOpType
    AF = mybir.ActivationFunctionType
    AX = mybir.AxisListType
    tiles = meta['tiles']; colmap = meta['colmap']
    call_cols = meta['call_cols']; call_off = meta['call_off']
    gcol_off = meta['gcol_off']; ncols = meta['ncols']; gcols = meta['gcols']
    GMAX = int(call_cols.max())
    CWMAX = 0
    for w in range(NW):
        CWMAX = max(CWMAX, int(call_off[w, NG - 1] + call_cols[w, NG - 1]
                               - call_off[w, 0]))

    with tc.tile_pool(name="persist", bufs=1) as pp:
        iotab = pp.tile([128, 128], bf, tag="iotab")
        nc.sync.dma_start(iotab[:], H['iota128'][:])
        identb = pp.tile([128, 128], bf, tag="identb")
        nc.sync.dma_start(identb[:], H['ident'][:])
        onesb = pp.tile([1, 128], bf, tag="onesb")
        nc.vector.memset(onesb[:], 1.0)
        sel64b = pp.tile([128, 64], bf, tag="sel64b")
        nc.sync.dma_start(sel64b[:], H['sel64'][:])
        tcolh = pp.tile([128, ncols], bf, tag="tcolh")
        nc.sync.dma_start(tcolh[:], H['tcol'][:])
        maskh = pp.tile([128, ncols], f32, tag="maskh")
        nc.sync.dma_start(maskh[:], H['mask'][:])
        idxsb = []
        msk8b = []
        for g in range(NG):
            ix = pp.tile([128, gcols[g] * 8], dt.int16, tag=f"idx{g}")
            nc.sync.dma_start(ix[:], H['idx'][g][:])
            idxsb.append(ix)
            m8 = pp.tile([128, gcols[g] * 8], bf, tag=f"m8_{g}")
            nc.sync.dma_start(m8[:], H['msk8'][g][:])
            msk8b.append(m8)
        hT1 = pp.tile([C1, Nc], bf, tag="hT1")
        hT2 = pp.tile([C2, Nc], bf, tag="hT2")

        for lay in (1, 2):
            C = C1 if lay == 1 else C2
            ROWE = ROW1 if lay == 1 else ROW2
            W0 = 1 + C                  # [1 | h] width (whj slice)
            Wl = 1 + C + 8              # written row words
            hT = hT1 if lay == 1 else hT2
            hx = H['hx1'] if lay == 1 else H['hx2']
            zT = H['zT1'] if lay == 1 else H['zT2']
            Fd = H['F1'] if lay == 1 else H['F2']
            Wcat = H['Wcat1'] if lay == 1 else H['Wcat2']
            attiT = H['attiT1'] if lay == 1 else H['attiT2']
            Wq = H['Wq1'] if lay == 1 else H['Wq2']
            Wk = H['Wk1'] if lay == 1 else H['Wk2']
            Wv = H['Wv1'] if lay == 1 else H['Wv2']
            Ws = H['Ws1'] if lay == 1 else H['Ws2']
            CIN = 128

            if lay == 1:
                abatches = [(H['xT'][:, k0:k0 + min(512, N - k0)], k0,
                             min(512, N - k0)) for k0 in range(0, N, 512)]
            else:
                abatches = []
                for g in range(NCORES):
                    for k0 in range(0, Nc, 512):
                        tot = min(512, Nc - k0)
                        abatches.append(
                            (H['x2T'][g * 128:(g + 1) * 128, k0:k0 + tot],
                             g * Nc + k0, tot))

            # ---------------- Phase A ----------------
            with tc.tile_pool(name="pa", bufs=3) as pa, \
                 tc.tile_pool(name="pap", bufs=2, space="PSUM") as pap:
                wcat = pa.tile([CIN, C + 8], bf, tag="wcat")
                nc.sync.dma_start(wcat[:], Wcat[:])
                for (src_ap, n0, tot) in abatches:
                    nt = (tot + 127) // 128
                    xt_sb = pa.tile([CIN, 512], bf, tag="xt")
                    nc.sync.dma_start(xt_sb[:, :tot], src_ap)
                    # 256-f32 stride per j keeps each matmul out inside one
                    # PSUM bank
                    hps4 = pap.tile([128, 4 * 256], f32, tag="hps4")
                    stg = pa.tile([128, 4 * Wl], bf, tag="stg")
                    stg3 = stg[:, 0:nt * Wl].rearrange("p (j w) -> p j w", w=Wl)
                    nc.vector.memset(stg3[:, :, 0:1], 1.0)
                    for j in range(nt):
                        nn = min(128, tot - j * 128)
                        nc.tensor.matmul(
                            hps4[:nn, j * 256:j * 256 + C + 8],
                            xt_sb[:, j * 128:j * 128 + nn],
                            wcat[:], start=True, stop=True)
                    h43 = hps4[:, 0:nt * 256].rearrange(
                        "p (j w) -> p j w", w=256)
                    nc.vector.tensor_copy(stg3[:, :, 1:Wl], h43[:, :, 0:C + 8])
                    nfull = tot // 128
                    rem = tot - nfull * 128
                    if nfull:
                        nc.sync.dma_start(
                            bass.AP(hx.tensor, n0 * ROWE,
                                    [[ROWE, 128], [ROWE * 128, nfull],
                                     [1, Wl]]),
                            stg[:, 0:nfull * Wl].rearrange(
                                "p (j w) -> p j w", w=Wl))
                    if rem:
                        nc.sync.dma_start(
                            hx[n0 + nfull * 128:n0 + tot, 0:Wl],
                            stg[:rem, nfull * Wl:(nfull + 1) * Wl])
                # hT_local + F table
                atti = pa.tile([C, 8], bf, tag="atti")
                nc.sync.dma_start(atti[:], attiT[:])
                for k in range((Nc + 127) // 128):
                    n0 = k * 128
                    nn = min(128, Nc - n0)
                    xt2 = pa.tile([CIN, 128], bf, tag="xt2")
                    if lay == 1:
                        src_loc = H['xTloc'][:, n0:n0 + nn]
                    else:
                        src_loc = H['o1T'][:, n0:n0 + nn]
                    nc.sync.dma_start(xt2[:, :nn], src_loc)
                    hpsl = pap.tile([128, 128], f32, tag="hpsl", bufs=1)
                    nc.tensor.matmul(hpsl[:C, :nn], wcat[:, 0:C], xt2[:, :nn],
                                     start=True, stop=True)
                    nc.vector.tensor_copy(hT[:C, n0:n0 + nn], hpsl[:C, :nn])
                    sps = pap.tile([8, 128], f32, tag="sps", bufs=1)
                    nc.tensor.matmul(sps[:8, :nn], atti[:C, :],
                                     hT[:C, n0:n0 + nn], start=True, stop=True)
                    fts = pa.tile([8, 128], bf, tag="fts")
                    nc.scalar.activation(fts[:8, :nn], sps[:8, :nn],
                                         AF.Exp, scale=0.5)
                    ftp = pap.tile([128, 8], bf, tag="ftp", bufs=1)
                    nc.tensor.transpose(ftp[:nn, :], fts[:8, :nn],
                                        identb[0:8, 0:8])
                    ftr = pa.tile([128, 8], bf, tag="ftr")
                    nc.vector.tensor_copy(ftr[:nn, :], ftp[:nn, :])
                    nc.sync.dma_start(
                        bass.AP(Fd.tensor, n0 * 8, [[8, nn], [1, 8]]),
                        ftr[:nn, :])
                nc.sync.dma_start(Fd[RNc:RNc_pad],
                                  onesb[0:1, 0:RNc_pad - RNc])

            if _stop == f"A{lay}":
                return
            # ---------------- Phase B: edges ----------------
            with tc.tile_pool(name="pb", bufs=2) as pb, \
                 tc.tile_pool(name="pbs", bufs=2) as pbs, \
                 tc.tile_pool(name="sfp", bufs=4 * WBLK + 8) as sfp, \
                 tc.tile_pool(name="whp", bufs=6) as whp, \
                 tc.tile_pool(name="frp", bufs=1, space="PSUM") as frp, \
                 tc.tile_pool(name="zpp", bufs=3, space="PSUM") as zpp, \
                 tc.tile_pool(name="ztq", bufs=2, space="PSUM") as ztq:
                for w in range(NW):
                    b0 = w * WBLK
                    b1 = min(b0 + WBLK, NBLK)
                    nb = b1 - b0
                    gc0 = int(call_off[w, 0])
                    gc1 = int(call_off[w, NG - 1] + call_cols[w, NG - 1])
                    cw = gc1 - gc0
                    stgs = []
                    for g in range(NG):
                        cols = int(call_cols[w, g])
                        io = int(gcol_off[w, g])
                        gt = pb.tile([128, GMAX * ROWE], bf, tag=f"G{g}")
                        nc.gpsimd.dma_gather(
                            gt[:, 0:cols * ROWE].rearrange(
                                "p (c e) -> p c e", e=ROWE),
                            hx[SPLITS[g]:SPLITS[g + 1], 0:ROWE],
                            idxsb[g][:, io * 8:(io + cols) * 8],
                            cols * 128, cols * 128, ROWE)
                        stgs.append(gt)
                    fsb = pbs.tile([1, WBLK * TB], bf, tag="fsb")
                    nc.sync.dma_start(fsb[0:1, 0:nb * TB], Fd[b0 * TB:b1 * TB])
                    frep = frp.tile([128, WBLK * TB], f32, tag="frep")
                    # split at 512 f32 so each matmul out stays in one bank
                    for h0 in range(0, nb * TB, 512):
                        h1 = min(h0 + 512, nb * TB)
                        nc.tensor.matmul(frep[:, h0:h1], onesb[:],
                                         fsb[0:1, h0:h1],
                                         start=True, stop=True)
                    frepb = pbs.tile([128, WBLK * TB], bf, tag="frepb")
                    nc.scalar.activation(frepb[:, 0:nb * TB],
                                         frep[:, 0:nb * TB], AF.Copy)
                    siFw = pbs.tile([128, CWMAX], f32, tag="siFw")
                    sfacs = {}
                    for b in range(b0, b1):
                        for g in range(NG):
                            for k in range(int(tiles[b, g])):
                                gc = int(call_off[w, g]) + colmap[(b, g, k)]
                                sf = sfp.tile([128, TB], bf, tag="sfac")
                                nc.vector.scalar_tensor_tensor(
                                    sf[:], iotab[:], tcolh[:, gc:gc + 1],
                                    frepb[:, (b - b0) * TB:(b - b0 + 1) * TB],
                                    A.is_equal, A.mult,
                                    accum_out=siFw[:, gc - gc0:gc - gc0 + 1])
                                sfacs[(b, g, k)] = sf
                    # sj selection per group
                    sjsel = pbs.tile([128, CWMAX], bf, tag="sjsel")
                    for g in range(NG):
                        cols = int(call_cols[w, g])
                        o = int(call_off[w, g]) - gc0
                        io = int(gcol_off[w, g])
                        G3 = stgs[g][:, 0:cols * ROWE].rearrange(
                            "p (c e) -> p c e", e=ROWE)
                        s8 = pbs.tile([128, GMAX * 8], bf, tag="s8")
                        s83 = s8[:, 0:cols * 8].rearrange(
                            "p (c e) -> p c e", e=8)
                        nc.vector.tensor_tensor(
                            s83,
                            msk8b[g][:, io * 8:(io + cols) * 8].rearrange(
                                "p (c e) -> p c e", e=8),
                            G3[:, :, W0:W0 + 8], A.mult)
                        nc.vector.tensor_reduce(
                            sjsel[:, o:o + cols], s83, AX.X, A.add)
                    # alpha chain (fp32)
                    calp = pbs.tile([128, CWMAX], f32, tag="calp")
                    nc.scalar.activation(calp[:, 0:cw], siFw[:, 0:cw], AF.Ln)
                    nc.vector.scalar_tensor_tensor(
                        calp[:, 0:cw], calp[:, 0:cw], 2.0, sjsel[:, 0:cw],
                        A.mult, A.add)
                    nc.vector.scalar_tensor_tensor(
                        calp[:, 0:cw], calp[:, 0:cw], 0.2, calp[:, 0:cw],
                        A.mult, A.max)
                    nc.vector.scalar_tensor_tensor(
                        calp[:, 0:cw], calp[:, 0:cw], 80.0,
                        maskh[:, gc0:gc1], A.min, A.add)
                    warrw = pbs.tile([128, CWMAX], f32, tag="warrw")
                    nc.scalar.activation(warrw[:, 0:cw], calp[:, 0:cw], AF.Exp)
                    # weighting + z matmuls + per-block normalize/transpose
                    znw = pbs.tile([128, WBLK * C], bf, tag="znw")
                    drec = pbs.tile([128, WBLK], f32, tag="drec")
                    ztw = ztq.tile([C, WBLK * TB], bf, tag="ztw")
                    for b in range(b0, b1):
                        nt = [(g, k) for g in range(NG)
                              for k in range(int(tiles[b, g]))]
                        zps = zpp.tile([128, 1 + C], f32, tag="zps")
                        for i, (g, k) in enumerate(nt):
                            gc = int(call_off[w, g]) + colmap[(b, g, k)]
                            c = colmap[(b, g, k)]
                            G3 = stgs[g][:, 0:int(call_cols[w, g]) * ROWE] \
                                .rearrange("p (c e) -> p c e", e=ROWE)
                            whj = whp.tile([128, 1 + C], bf, tag="whj")
                            wsc = warrw[:, gc - gc0:gc - gc0 + 1]
                            if gc % 2 == 0:
                                nc.scalar.activation(
                                    whj[:], G3[:, c, 0:W0], AF.Copy,
                                    scale=wsc)
                            else:
                                nc.vector.tensor_scalar_mul(
                                    whj[:], G3[:, c, 0:W0], wsc)
                            nc.tensor.matmul(zps[:], sfacs[(b, g, k)][:],
                                             whj[:], start=(i == 0),
                                             stop=(i == len(nt) - 1))
                        db = drec[:, b - b0:b - b0 + 1]
                        nc.vector.tensor_scalar(db, zps[:, 0:1], EPS, None,
                                                A.add)
                        nc.vector.reciprocal(db, db)
                        nc.vector.tensor_scalar_mul(
                            znw[:, (b - b0) * C:(b - b0 + 1) * C],
                            zps[:, 1:1 + C], db)
                        nc.tensor.transpose(
                            ztw[:C, (b - b0) * TB:(b - b0 + 1) * TB],
                            znw[:, (b - b0) * C:(b - b0 + 1) * C], identb[:])
                    zsb = pbs.tile([C, WBLK * TB], bf, tag="zsb")
                    nc.vector.tensor_copy(zsb[:C, 0:nb * TB],
                                          ztw[:C, 0:nb * TB])
                    nc.sync.dma_start(zT[:C, b0 * TB:b1 * TB],
                                      zsb[:C, 0:nb * TB])

            if _stop == f"B{lay}":
                return
            # ---------------- Phase C: tail ----------------
            with tc.tile_pool(name="pc", bufs=3) as pc, \
                 tc.tile_pool(name="pcw", bufs=1) as pcw, \
                 tc.tile_pool(name="pck", bufs=1, space="PSUM") as pck, \
                 tc.tile_pool(name="pcq", bufs=1, space="PSUM") as pcq, \
                 tc.tile_pool(name="pct", bufs=1, space="PSUM") as pct:
                wq = pcw.tile([C, 8 * C], bf, tag="wq")
                nc.sync.dma_start(wq[:], Wq[:])
                wk = pcw.tile([C, 8 * C], bf, tag="wk")
                nc.sync.dma_start(wk[:], Wk[:])
                wv = pcw.tile([C, 8 * C], bf, tag="wv")
                nc.sync.dma_start(wv[:], Wv[:])
                ws = pcw.tile([C, C], bf, tag="ws")
                nc.sync.dma_start(ws[:], Ws[:])
                for it in range((Nc + 127) // 128):
                    n0 = it * 128
                    nn = min(128, Nc - n0)
                    ztc = pc.tile([C, 1024], bf, tag="ztc")
                    nc.sync.dma_start(ztc[:C, 0:nn * 8],
                                      zT[:C, n0 * 8:(n0 + nn) * 8])
                    zt3 = ztc[:C, 0:nn * 8].rearrange("c (n r) -> c r n", r=8)
                    kps = pck.tile([C, 128], f32, tag="kps")
                    vps = pck.tile([C, 128], f32, tag="vps")
                    for r in range(8):
                        nc.tensor.matmul(kps[:C, :nn],
                                         wk[:, r * C:(r + 1) * C],
                                         zt3[:, r, :], start=(r == 0),
                                         stop=(r == 7))
                    for r in range(8):
                        nc.tensor.matmul(vps[:C, :nn],
                                         wv[:, r * C:(r + 1) * C],
                                         zt3[:, r, :], start=(r == 0),
                                         stop=(r == 7))
                    ks = pc.tile([C, 128], bf, tag="ks")
                    nc.scalar.activation(ks[:C, :nn], kps[:C, :nn], AF.Copy)
                    vsb = pc.tile([C, 128], bf, tag="vsb")
                    nc.scalar.activation(vsb[:C, :nn], vps[:C, :nn], AF.Copy)
                    qps = pcq.tile([C, 1024], f32, tag="qps")
                    for r in range(8):
                        nc.tensor.matmul(qps[:C, r * 128:r * 128 + nn],
                                         wq[:, r * C:(r + 1) * C],
                                         zt3[:, r, :], start=True, stop=True)
                    qsb = pc.tile([C, 1024], bf, tag="qsb")
                    q3p = qps[:C, 0:1024].rearrange(
                        "c (r n) -> c r n", n=128)[:, :, 0:nn]
                    q3s = qsb[:C, 0:1024].rearrange(
                        "c (r n) -> c r n", n=128)[:, :, 0:nn]
                    nc.scalar.activation(q3s, q3p, AF.Copy)
                    tmp = pc.tile([C, 1024], bf, tag="tmp")
                    t3 = tmp[:C, 0:1024].rearrange(
                        "c (r n) -> c r n", n=128)[:, :, 0:nn]
                    nc.vector.tensor_tensor(
                        t3, q3s,
                        ks[:C, :nn].unsqueeze(1).to_broadcast((C, 8, nn)),
                        A.mult)
                    pps = pck.tile([8, 128], f32, tag="pps")
                    for r in range(8):
                        nc.tensor.matmul(pps[:8, :nn],
                                         sel64b[:C, r * 8:(r + 1) * 8],
                                         tmp[:C, r * 128:r * 128 + nn],
                                         start=(r == 0), stop=(r == 7))
                    psb = pc.tile([8, 128], bf, tag="psb")
                    nc.vector.tensor_copy(psb[:8, :nn], pps[:8, :nn])
                    ptp = pct.tile([128, 8], bf, tag="tp8")
                    nc.tensor.transpose(ptp[:nn, :], psb[:8, :nn],
                                        identb[0:8, 0:8])
                    psiT = pc.tile([128, 8], bf, tag="psiT")
                    nc.vector.tensor_copy(psiT[:nn, :], ptp[:nn, :])
                    vtp = pct.tile([128, 128], bf, tag="tpC")
                    nc.tensor.transpose(vtp[:nn, :C], vsb[:C, :nn],
                                        identb[0:C, 0:C])
                    vsum = pc.tile([128, 128], bf, tag="vsum")
                    nc.vector.tensor_copy(vsum[:nn, :C], vtp[:nn, :C])
                    bps = pcq.tile([128, 128], f32, tag="bps")
                    nc.tensor.matmul(bps[:nn, :C], hT[:C, n0:n0 + nn], ws[:],
                                     start=True, stop=True)
                    bsb = pc.tile([128, 128], bf, tag="bsb")
                    nc.scalar.activation(bsb[:nn, :C], bps[:nn, :C], AF.Copy)
                    # delta softmax over channels, per relation
                    g2 = pc.tile([128, 8 * C], bf, tag="g2")
                    for r in range(8):
                        nc.vector.scalar_tensor_tensor(
                            g2[:nn, r * C:(r + 1) * C], vsum[:nn, :C],
                            psiT[:nn, r:r + 1], bsb[:nn, :C],
                            A.mult, A.add)
                    g23 = g2[:nn, :].rearrange("p (r c) -> p r c", r=8)
                    m8 = pc.tile([128, 8], bf, tag="m8")
                    nc.vector.tensor_reduce(m8[:nn, :], g23, AX.X, A.max)
                    m8f = pc.tile([128, 8], f32, tag="m8f")
                    nc.vector.tensor_copy(m8f[:nn, :], m8[:nn, :])
                    for r in range(8):
                        nc.vector.tensor_scalar_sub(
                            g2[:nn, r * C:(r + 1) * C],
                            g2[:nn, r * C:(r + 1) * C], m8f[:nn, r:r + 1])
                    nc.scalar.activation(g2[:nn, :], g2[:nn, :], AF.Exp)
                    ssum = pc.tile([128, 8], bf, tag="ssum")
                    nc.vector.tensor_reduce(ssum[:nn, :], g23, AX.X, A.add)
                    srec = pc.tile([128, 8], f32, tag="srec")
                    nc.vector.reciprocal(srec[:nn, :], ssum[:nn, :])
                    outc = pc.tile([128, 128], bf, tag="outc")
                    nc.vector.tensor_scalar_mul(outc[:nn, :C], g2[:nn, 0:C],
                                                srec[:nn, 0:1])
                    for r in range(1, 8):
                        nc.vector.scalar_tensor_tensor(
                            outc[:nn, :C], g2[:nn, r * C:(r + 1) * C],
                            srec[:nn, r:r + 1], outc[:nn, :C],
                            A.mult, A.add)
                    if lay == 1:
                        otp = pct.tile([128, 128], bf, tag="tpC")
                        nc.tensor.transpose(otp[:C, :nn], outc[:nn, :C],
                                            identb[0:nn, 0:nn])
                        o1s = pc.tile([C, 128], bf, tag="o1s")
                        nc.vector.tensor_copy(o1s[:C, :nn], otp[:C, :nn])
                        nc.sync.dma_start(H['o1T'][:, n0:n0 + nn],
                                          o1s[:C, :nn])
                    else:
                        m1 = pc.tile([128, 1], f32, tag="m1")
                        nc.vector.tensor_reduce(m1[:nn, :], outc[:nn, :C],
                                                AX.X, A.max)
                        nc.vector.tensor_scalar_mul(m1[:nn, :], m1[:nn, :],
                                                    -1.0)
                        sc = pc.tile([128, 64], f32, tag="sc")
                        s1 = pc.tile([128, 1], f32, tag="s1")
                        nc.scalar.activation(sc[:nn, :], outc[:nn, :C],
                                             AF.Exp, bias=m1[:nn, :],
                                             accum_out=s1[:nn, :])
                        lns = pc.tile([128, 1], f32, tag="lns")
                        nc.scalar.activation(lns[:nn, :], s1[:nn, :], AF.Ln)
                        res = pc.tile([128, 64], f32, tag="res")
                        nc.vector.scalar_tensor_tensor(
                            res[:nn, :], outc[:nn, :C], m1[:nn, :],
                            lns[:nn, :].to_broadcast((nn, C)),
                            A.add, A.subtract)
                        nc.sync.dma_start(H['out'][n0:n0 + nn, :],
                                          res[:nn, :])
            if _stop == f"C{lay}":
                return
            if lay == 1:
                nc.gpsimd.collective_compute(
                    "AllGather", A.bypass,
                    replica_groups=[list(range(NCORES))],
                    ins=[H['o1T'][:]],
                    outs=[H['x2T'][:]])


def _build_module(bass, bacc, mybir, tile, meta):
    ncols = meta['ncols']
    gcols = meta['gcols']

    f32 = mybir.dt.float32
    bf = mybir.dt.bfloat16
    i16 = mybir.dt.int16
    nc = bacc.Bacc("TRN2", target_bir_lowering=False, debug=False,
                   num_devices=NCORES)

    def din(name, shape, dtype=bf):
        return nc.dram_tensor(name, list(shape), dtype,
                              kind="ExternalInput").ap()

    H = {}
    H['xT'] = din("xT", [128, N])
    H['xTloc'] = din("xTloc", [128, Nc])
    H['iota128'] = din("iota128", [128, 128])
    H['ident'] = din("ident", [128, 128])
    H['sel64'] = din("sel64", [128, 64])
    H['tcol'] = din("tcol", [128, ncols])
    H['mask'] = din("mask", [128, ncols], f32)
    H['idx'] = [din(f"idx{g}", [128, gcols[g] * 8], i16) for g in range(NG)]
    H['msk8'] = [din(f"msk8_{g}", [128, gcols[g] * 8]) for g in range(NG)]
    for l, c in ((1, C1), (2, C2)):
        H[f'Wcat{l}'] = din(f"Wcat{l}", [128, c + 8])
        H[f'attiT{l}'] = din(f"attiT{l}", [c, 8])
        H[f'Wq{l}'] = din(f"Wq{l}", [c, 8 * c])
        H[f'Wk{l}'] = din(f"Wk{l}", [c, 8 * c])
        H[f'Wv{l}'] = din(f"Wv{l}", [c, 8 * c])
        H[f'Ws{l}'] = din(f"Ws{l}", [c, c])
    H['hx1'] = nc.dram_tensor("hx1", [N, ROW1], bf).ap()
    H['hx2'] = nc.dram_tensor("hx2", [N, ROW2], bf).ap()
    H['zT1'] = nc.dram_tensor("zT1", [C1, RNc_pad], bf).ap()
    H['zT2'] = nc.dram_tensor("zT2", [C2, RNc_pad], bf).ap()
    H['F1'] = nc.dram_tensor("F1", [RNc_pad], bf).ap()
    H['F2'] = nc.dram_tensor("F2", [RNc_pad], bf).ap()
    H['o1T'] = nc.dram_tensor("o1T", [128, Nc], bf).ap()
    H['x2T'] = nc.dram_tensor("x2T", [NCORES * 128, Nc], bf,
                              addr_space="Shared").ap()
    H['out'] = nc.dram_tensor("out", [Nc, C2], f32,
                              kind="ExternalOutput").ap()

    with tile.TileContext(nc) as tc, \
         nc.allow_low_precision(reason="bf16 validated: rel err ~4e-3 < 2e-2"):
        _emit(nc, tc, bass, mybir, meta, H)
    nc.compile()
    return nc, H


def kernel(**inputs):
    import concourse.bass as bass
    import concourse.bacc as bacc
    import concourse.mybir as mybir
    import concourse.tile as tile
    from concourse.bass_utils import run_bass_kernel_spmd

    ins = {k: np.asarray(v) for k, v in inputs.items()}
    tabs, meta = _host_prep(ins['edge_index'], ins['edge_type'])
    nc, H = _build_module(bass, bacc, mybir, tile, meta)

    BF = mybir.dt.np(mybir.dt.bfloat16)

    def tb(a):
        return np.ascontiguousarray(a).astype(BF)

    # host-side constant inputs
    x = ins['x'].astype(np.float32)
    iota128 = np.broadcast_to(np.arange(128, dtype=np.float32), (128, 128))
    ident = np.eye(128, dtype=np.float32)
    sel64 = np.zeros((128, 64), np.float32)
    for r in range(8):
        sel64[:, r * 8 + r] = 1.0

    common = dict(
        xT=tb(x.T),
        iota128=tb(iota128),
        ident=tb(ident), sel64=tb(sel64),
    )
    for l, c in ((1, C1), (2, C2)):
        att = ins[f'att{l}'].astype(np.float32)
        Wn = ins[f'Wn{l}'].astype(np.float32)
        common[f'Wcat{l}'] = tb(
            np.concatenate([Wn, Wn @ att[:, c:].T], axis=1))
        common[f'attiT{l}'] = tb(att[:, :c].T)
        for nm in ('Wq', 'Wk', 'Wv'):
            W = ins[f'{nm}{l}'].astype(np.float32)
            common[f'{nm}{l}'] = tb(
                W.transpose(1, 0, 2).reshape(c, 8 * c))
        common[f'Ws{l}'] = tb(ins[f'Ws{l}'])

    in_maps = []
    for m in range(NCORES):
        im = dict(common)
        im['xTloc'] = tb(x.T[:, m * Nc:(m + 1) * Nc])
        im['tcol'] = tb(tabs[m]['tcol'])
        im['mask'] = tabs[m]['mask']
        for g in range(NG):
            im[f'idx{g}'] = tabs[m]['idx'][g]
            im[f'msk8_{g}'] = tb(tabs[m]['msk8'][g])
        in_maps.append(im)

    global _LAST_BUILD, LAST_EXEC_NS
    _LAST_BUILD = (nc, in_maps)
    res = run_bass_kernel_spmd(nc, in_maps, list(range(NCORES)))
    LAST_EXEC_NS = res.exec_time_ns
    out = np.concatenate([res.results[m]['out'] for m in range(NCORES)], 0)
    return out.astype(np.float32)


LAST_EXEC_NS = None
_LAST_BUILD = None
